# revision 32
# baseline (speedup 1.0000x reference)
"""WENO5 2D advection (Advection3D) Trainium2 kernel — bf16 compute with
fp32 flux tail.

Full inputs h, u, v: [32, 1024, 1024] f32.  Output: same shape f32;
out[1:-1, 2:-2, 2:-2] = -div(WENO5 fluxes), 0 on the frame.

Sharding: z-levels across 8 cores (pure data parallel, no halo in z).
Per-core SPMD program processes ZPC=4 z-levels; each z-level is swept in
y-chunks of 128 rows (122 valid output rows per chunk).

Perf design (fp32 baseline ~112 us/chunk -> ~55 us/chunk):
  - h is bf16 in SBUF; the WENO smoothness/weight chain runs in bf16 so
    DVE tensor_tensor hits 2x_1P packed mode (elements step 1, every AP
    4B-aligned -> all windows use even column offsets; odd-offset stencil
    reads go through shifted copies, and the x-direction R-side chain is
    stored at a +1 column offset).
  - scalar_tensor_tensor has no 2x mode: eliminated.  Scalars fold into
    ACT scale/bias (squares absorb 0.5/C1312S; 5/6 rides Exp bias as
    ln(5/6)), or pre-scaled D-variants via tensor_scalar (2x_2P).
  - Newton reciprocal step dropped (ACT ln/exp LUT is ~2ulp fp32).
  - PE band matmuls in bf16; the y-divergence fn_{p-1}-fn_p is one band
    (DFY, fp32 copy for the fp32 fn), read once from PSUM by the final
    combine.
  - Engine balancing: squares run on GpSimd (tensor_tensor self-mult /
    scalar_tensor_tensor), QS/DS shift copies on SBUF->SBUF DMA, other
    shifts on ACT.  x/y chains are emitted interleaved so each engine's
    in-order queue always holds ready work.
  - Accuracy: u, v stay fp32; reconstruction outputs rL/rR and the whole
    flux tail (aa, bb, fe, fn, z1, out) are fp32 (these carry the
    output-scale values; bf16 would round each at ~0.4%).

Math (per face i, L stored at i, R stored at i+1 ("primed"); D_j =
q_{j+1}-q_j, A_j = D_j - D_{j-1}):
  G0 = c1312 A^2 + (0.5A + D)^2        (Sq of t0h)
  G1 = c1312 A^2 + (0.5(D+DS))^2       (Sq of sh)
  G2 = c1312 A^2 + (0.5A - DS)^2       (Sq of t1h)
  B_k = (eps + G_k)^2 ; PP12 = B1*B2S, PP01 = B0S*B1, PP02 = B0S*B2S
  denL10 = PP12 + 6 PP02 + 3 PP01 ; denR10 = PP01 + 6 PP02 + 3 PP12
  rdL = (5/6)/denL10 = Exp(-Ln(denL10) + ln(5/6))
  numL12 = PP12*dl0L + PP02*(2.4 dl1L) + PP01*(2.4 dl2L)
  qL = q_i + numL12*rdL ; qR' = q_j - numR12'*rdR'   (j = i+1)
  flux = relu(U)*qL - relu(-U)*qR
"""
import math

import numpy as np

import concourse.bass as bass
import concourse.mybir as mybir
import concourse.tile as tile

F32 = mybir.dt.float32
BF16 = mybir.dt.bfloat16
ALU = mybir.AluOpType
AF = mybir.ActivationFunctionType

NZ, NY, NX = 32, 1024, 1024
NCORES = 8
ZPC = 4                      # z-levels per core (SPMD-uniform)
PY, PX = NY + 2, NX + 2      # edge-padded input
W = 1032                     # tile width; data at cols [2:1028) <-> padded [0:1026)
DX = 1000.0
DY = 1000.0
WENO_EPS = 1e-6
C1312 = 13.0 / 12.0
C1312S = math.sqrt(C1312)
LN56 = math.log(5.0 / 6.0)
CHUNK = 122                  # valid output rows per 128-row chunk
USE_POOL = False             # GpSimd compute ops (slow sw handlers on HW?)


class LegalTileContext(tile.TileContext):
    """Tile + wait legalization: this walrus packs at most ONE semaphore wait
    per instruction; hoist extras onto standalone EventSemaphore instructions
    (what raw-bass wait_ge emits)."""

    def _commit_instruction(self, inst, lazy_reg_writes=True):
        si = inst.sync_info
        if si is not None and len(si.on_wait) > 1:
            waits = list(si.on_wait)
            for w in waits[:-1]:
                ev = mybir.InstEventSemaphore(
                    name=f"W-{self.nc.next_id()}", ins=[], outs=[]
                )
                ev.engine = inst.engine
                ev.sync_info = mybir.SyncInfo(on_wait=[w], on_update=[])
                if inst.debug is not None:
                    ev.debug = inst.debug
                super()._commit_instruction(ev, lazy_reg_writes=False)
            inst.sync_info = mybir.SyncInfo(
                on_wait=[waits[-1]], on_update=list(si.on_update)
            )
        return super()._commit_instruction(inst, lazy_reg_writes)

    def _drain_and_barrier(self, tick_clock, wait_clock):
        from concourse.vector_clock import ScopedClock

        nop0 = self.nc.sync.nop()
        wait_clock.add_sem_waits(
            nop0.ins, ScopedClock({None: tick_clock.global_clock})
        )
        si = nop0.ins.sync_info
        if si is not None and len(si.on_wait) > 1:
            waits = list(si.on_wait)
            nop0.ins.sync_info = mybir.SyncInfo(
                on_wait=[waits[0]], on_update=list(si.on_update)
            )
            for w in waits[1:]:
                nopk = self.nc.sync.nop()
                nopk.ins.sync_info = mybir.SyncInfo(on_wait=[w], on_update=[])
        self.nc.sync.drain()

        self.nc.all_engine_barrier()
        assert self.sems is not None
        popped = self.nc._tile_sem_poison_stack.pop()
        assert popped is self._sem_poison
        self.nc.clear_and_free_semaphores(list(self.sems.allocated().values()))
        self.nc.all_engine_barrier()


class Scratch:
    """Free-list scratch allocator.  Tags are reused only after an explicit
    free(), which callers place after the tile's last consumer is emitted —
    so slot-wait edges always point backward in emission order and can
    never form a scheduling cycle."""

    def __init__(self, pool, shape, dtype, prefix="s"):
        self.pool = pool
        self.shape = shape
        self.dtype = dtype
        self.prefix = prefix
        self.free_tags = []
        self.n = 0
        self.tag_of = {}

    def __call__(self):
        # FIFO reuse: freed tags get maximal cool-down before their buffers
        # are written again (fewer WAR slot waits than LIFO).
        tag = (
            self.free_tags.pop(0) if self.free_tags else f"{self.prefix}{self._new()}"
        )
        t = self.pool.tile(self.shape, self.dtype, tag=tag)
        self.tag_of[id(t)] = tag
        return t

    def _new(self):
        self.n += 1
        return self.n - 1

    def free(self, *tiles):
        for t in tiles:
            self.free_tags.append(self.tag_of.pop(id(t)))


# Band matrices (lhsT layout: S[k, p] = coeff of q_k in out_p), bf16.
BAND_SPECS = [
    ("shp1", {1: 1.0}),                        # 0: out_p = q_{p+1}
    ("ay", {-1: 1.0, 0: -2.0, 1: 1.0}),        # 1: A_p
    ("t0h", {-1: 0.5, 0: -2.0, 1: 1.5}),       # 2: 0.5*A + D
    ("t1h", {-1: 1.5, 0: -2.0, 1: 0.5}),       # 3: 0.5*A - DS
    ("sh", {-1: -0.5, 1: 0.5}),                # 4: 0.5*(D + DS)
    ("dl0L", {-2: 0.4, -1: -1.4, 0: 1.0}),     # 5
    ("dl1Lh", {-1: -1.2, 0: -1.2, 1: 2.4}),    # 6: 2.4*dl1L
    ("dl2Lh", {0: -2.4, 1: 3.0, 2: -0.6}),     # 7: 2.4*dl2L
    ("dl0R", {1: -1.0, 2: 1.4, 3: -0.4}),      # 8
    ("dl1Rh", {0: -2.4, 1: 1.2, 2: 1.2}),      # 9: 2.4*dl1R
    ("dl2Rh", {-1: 0.6, 0: -3.0, 1: 2.4}),     # 10: 2.4*dl2R
    ("shm1", {-1: 1.0}),                       # 11: out_p = q_{p-1}
    ("i1", {0: 1.0}),                          # 12: identity (accumulate)
    ("i6", {0: 6.0}),                          # 13: 6x identity
    ("i3", {0: 3.0}),                          # 14: 3x identity
    ("i1312", {0: 13.0 / 12.0}),               # 15: (13/12)x identity
    ("i025", {0: 0.25}),                       # 16: 0.25x identity
]
SHP1, AY, T0H, T1H, SH = 0, 1, 2, 3, 4
DL0L, DL1LH, DL2LH, DL0R, DL1RH, DL2RH = 5, 6, 7, 8, 9, 10
SHM1, I1, I6, I3, I1312, I025 = 11, 12, 13, 14, 15, 16
NBANDS = len(BAND_SPECS)
DFY_TAPS = {-1: 1.0, 0: -1.0}                  # fn_{p-1} - fn_p (fp32 band)


def _band_matrix(taps):
    w = np.zeros((128, 128), dtype=np.float32)
    for off, coef in taps.items():
        for p in range(128):
            k = p + off
            if 0 <= k < 128:
                w[k, p] = coef
    return w


def make_bands_host():
    """SBUF-layout band matrices: [128, NBANDS*128] bf16."""
    import ml_dtypes

    w = np.zeros((128, NBANDS * 128), dtype=np.float32)
    for b, (_, taps) in enumerate(BAND_SPECS):
        w[:, b * 128 : (b + 1) * 128] = _band_matrix(taps)
    return w.astype(ml_dtypes.bfloat16)


def make_dfy_host():
    return _band_matrix(DFY_TAPS)  # f32


E = slice(2, 1028)    # x-chain window (even start/len; data cols)
EY = slice(4, 1028)   # y-chain window (1024 cols = 2 PSUM banks)


def _emit_chunk(nc, sc, scf, psc, bands, dfy32, Q, Uf, Vf, oc2, mode="full"):
    """Emit one 128-row chunk, x/y chains interleaved.

    sc: bf16 scratch; scf: fp32 scratch (flux tail); psc: PSUM scratch.
    Q bf16; Uf, Vf fp32 (pre-scaled by 1/DX, 1/DY).  Result (fp32) is
    written to oc2; valid rows [3:125), cols [5:1025).

    Linear tile combinations (c_k = asq + q_k; den = PP + 6 PP + 3 PP)
    run on PE as accumulating identity-band matmuls into PSUM; eps is
    added via the Square bias when reading c back; the x B-shifts are
    folded into the PSUM->SBUF copies by writing at shifted offsets.
    """
    tt = nc.vector.tensor_tensor
    tsm = nc.vector.tensor_scalar_mul
    act = nc.scalar.activation
    gtt = nc.gpsimd.tensor_tensor
    gts = nc.gpsimd.tensor_scalar

    def pe(src, b, lo=4, bsrc=None):
        bsrc = bands if bsrc is None else bsrc
        pt = psc()
        for c0 in (0, 512):
            nc.tensor.matmul(
                pt[:, c0 : c0 + 512],
                bsrc[:, b * 128 : (b + 1) * 128],
                src[:, lo + c0 : lo + c0 + 512],
            )
        return pt

    def pe_acc(srcs_and_bands, lo):
        """PSUM-accumulated sum of band-stencils: sum_k band_k @ src_k."""
        pt = psc()
        n = len(srcs_and_bands)
        for c0 in (0, 512):
            for k, (src, b) in enumerate(srcs_and_bands):
                nc.tensor.matmul(
                    pt[:, c0 : c0 + 512],
                    bands[:, b * 128 : (b + 1) * 128],
                    src[:, lo + c0 : lo + c0 + 512],
                    start=(k == 0),
                    stop=(k == n - 1),
                )
        return pt

    def pecopy(src, b, func=AF.Copy, scale=1.0):
        p = pe(src, b)
        t = sc()
        act(t[:, EY], p[:, 0:1024], func, scale=scale)
        psc.free(p)
        return t

    full = mode == "full"
    do_x = mode in ("full", "xonly")
    do_y = mode in ("full", "yonly")
    XL = slice(4, 1026)   # x late-section window (after PP)

    # ---- y producers: PE band stencils + ACT copies (need only Q) ----
    if do_y:
        yqs1 = pecopy(Q, SHP1)
        yasq = pecopy(Q, AY, AF.Square, C1312S)
        yq0 = pecopy(Q, T0H, AF.Square)
        yq2 = pecopy(Q, T1H, AF.Square)
        yq1 = pecopy(Q, SH, AF.Square)
        ydl0L = pecopy(Q, DL0L)
        ydl1L = pecopy(Q, DL1LH)
        ydl2L = pecopy(Q, DL2LH)
        ydl0R = pecopy(Q, DL0R)
        ydl1R = pecopy(Q, DL1RH)
        ydl2R = pecopy(Q, DL2RH)

    # ---- x stencils: QS/DS via SBUF->SBUF DMA, diffs on DVE, squares on
    # GpSimd ----
    if do_x:
        xQS = sc(); nc.vector.tensor_copy(xQS[:, E], Q[:, 3:1029])
        xD = sc(); tt(xD[:, E], xQS[:, E], Q[:, E], ALU.subtract)
        xDS = sc(); nc.vector.tensor_copy(xDS[:, E], xD[:, 1:1027])
        xA = sc(); tt(xA[:, E], xD[:, E], xDS[:, E], ALU.subtract)
        xD05A = sc(); tsm(xD05A[:, E], xA[:, E], 0.5)
        xt0h = sc(); tt(xt0h[:, E], xD05A[:, E], xD[:, E], ALU.add)
        xt1h = sc(); tt(xt1h[:, E], xD05A[:, E], xDS[:, E], ALU.subtract)
        sc.free(xD05A)
        xs = sc(); tt(xs[:, E], xD[:, E], xDS[:, E], ALU.add)
        if USE_POOL:
            # squares on Pool: plain self-mults; 13/12 and 0.25 ride the
            # c-accumulation bands (I1312/I025)
            xasq = sc(); gtt(xasq[:, E], xA[:, E], xA[:, E], ALU.mult)
            xq0 = sc(); gtt(xq0[:, E], xt0h[:, E], xt0h[:, E], ALU.mult)
            xq1 = sc(); gtt(xq1[:, E], xs[:, E], xs[:, E], ALU.mult)
            xq2 = sc(); gtt(xq2[:, E], xt1h[:, E], xt1h[:, E], ALU.mult)
        else:
            xasq = sc(); act(xasq[:, E], xA[:, E], AF.Square, scale=C1312S)
            xq0 = sc(); act(xq0[:, E], xt0h[:, E], AF.Square)
            xq1 = sc(); act(xq1[:, E], xs[:, E], AF.Square, scale=0.5)
            xq2 = sc(); act(xq2[:, E], xt1h[:, E], AF.Square)
        sc.free(xA)
        sc.free(xt0h, xt1h, xs)
        # pre-scaled D variants (tensor_scalar, 2x_2P at any alignment)
        xD4 = sc(); tsm(xD4[:, E], xD[:, E], -0.4)
        xD4S = sc(); tsm(xD4S[:, E], xDS[:, E], -0.4)
        xD12 = sc(); tsm(xD12[:, E], xD[:, E], 1.2)
        xD12S = sc(); tsm(xD12S[:, E], xDS[:, E], 1.2)
        xD24 = sc(); tsm(xD24[:, E], xD[:, E], 2.4)
        xD24S = sc(); tsm(xD24S[:, E], xDS[:, E], 2.4)
        xD06 = sc(); tsm(xD06[:, E], xD[:, E], -0.6)
        xD06S = sc(); tsm(xD06S[:, E], xDS[:, E], -0.6)
        xdl0L = sc(); tt(xdl0L[:, E], xD4[:, 0:1026], xDS[:, E], ALU.add)
        xdl1L = sc(); tt(xdl1L[:, E], xD12S[:, E], xD24[:, E], ALU.add)
        xdl2L = sc(); tt(xdl2L[:, E], xD06S[:, 4:1030], xD24[:, E], ALU.add)
        xdl0R = sc(); tt(xdl0R[:, E], xD4S[:, 4:1030], xD[:, E], ALU.add)
        xdl1R = sc(); tt(xdl1R[:, E], xD12[:, E], xD24S[:, E], ALU.add)
        xdl2R = sc(); tt(xdl2R[:, E], xD06[:, 0:1026], xD24S[:, E], ALU.add)
        sc.free(xD4, xD4S, xD12, xD12S, xD24, xD24S, xD06, xD06S, xD, xDS, xQS)

    # ---- y: c = asq + q_k on PE (accumulate), B = Sq(c + eps) on ACT ----
    if do_y:
        ycp = pe_acc([(yasq, I1), (yq0, I1)], 4)
        yB0 = sc(); act(yB0[:, EY], ycp[:, 0:1024], AF.Square, bias=WENO_EPS)
        psc.free(ycp)
        ycp = pe_acc([(yasq, I1), (yq1, I1)], 4)
        yB1 = sc(); act(yB1[:, EY], ycp[:, 0:1024], AF.Square, bias=WENO_EPS)
        psc.free(ycp)
        ycp = pe_acc([(yasq, I1), (yq2, I1)], 4)
        yB2 = sc(); act(yB2[:, EY], ycp[:, 0:1024], AF.Square, bias=WENO_EPS)
        psc.free(ycp)
        sc.free(yasq, yq0, yq1, yq2)

    # ---- x: same, with the B shifts folded into the PSUM->SBUF writes
    # (c-psum col c <-> x col c+3) ----
    if do_x:
        IA = I1312 if USE_POOL else I1
        IQ1 = I025 if USE_POOL else I1
        xcp = pe_acc([(xasq, IA), (xq0, I1)], 3)
        xB0S = sc()  # xB0S[t] = B0[t-1]
        act(xB0S[:, 4:1028], xcp[:, 0:1024], AF.Square, bias=WENO_EPS)
        psc.free(xcp)
        xcp = pe_acc([(xasq, IA), (xq1, IQ1)], 3)
        xB1 = sc()
        act(xB1[:, 3:1027], xcp[:, 0:1024], AF.Square, bias=WENO_EPS)
        psc.free(xcp)
        xcp = pe_acc([(xasq, IA), (xq2, I1)], 3)
        xB2S = sc()  # xB2S[t] = B2[t+1]
        act(xB2S[:, 2:1026], xcp[:, 0:1024], AF.Square, bias=WENO_EPS)
        psc.free(xcp)
        sc.free(xasq, xq0, xq1, xq2)

    # ---- y: PP products (DVE), den on PE-accumulate, ln/exp (ACT) ----
    if do_y:
        yB0m1 = pecopy(yB0, SHM1)
        yB2p1 = pecopy(yB2, SHP1)
        sc.free(yB0, yB2)
        yPP12 = sc(); tt(yPP12[:, EY], yB1[:, EY], yB2p1[:, EY], ALU.mult)
        yPP01 = sc(); tt(yPP01[:, EY], yB0m1[:, EY], yB1[:, EY], ALU.mult)
        yPP02 = sc(); tt(yPP02[:, EY], yB0m1[:, EY], yB2p1[:, EY], ALU.mult)
        sc.free(yB1, yB0m1, yB2p1)
        yPP01p1 = pecopy(yPP01, SHP1)
        yPP02p1 = pecopy(yPP02, SHP1)
        yPP12p1 = pecopy(yPP12, SHP1)
        yg0L = sc(); tt(yg0L[:, EY], yPP12[:, EY], ydl0L[:, EY], ALU.mult)
        yg1L = sc(); tt(yg1L[:, EY], yPP02[:, EY], ydl1L[:, EY], ALU.mult)
        yg2L = sc(); tt(yg2L[:, EY], yPP01[:, EY], ydl2L[:, EY], ALU.mult)
        sc.free(ydl0L, ydl1L, ydl2L)
        ydp = pe_acc([(yPP12, I1), (yPP02, I6), (yPP01, I3)], 4)
        ylnL = sc(); act(ylnL[:, EY], ydp[:, 0:1024], AF.Ln)
        psc.free(ydp)
        yrdL = sc(); act(yrdL[:, EY], ylnL[:, EY], AF.Exp, bias=LN56, scale=-1.0)
        sc.free(ylnL)
        ydp = pe_acc([(yPP01, I1), (yPP02, I6), (yPP12, I3)], 4)
        ylnR = sc(); act(ylnR[:, EY], ydp[:, 0:1024], AF.Ln)
        psc.free(ydp)
        yrdR = sc(); act(yrdR[:, EY], ylnR[:, EY], AF.Exp, bias=LN56, scale=-1.0)
        sc.free(ylnR)

    # ---- x: PP products, den on PE (psum col c <-> x col c+2), ln/exp ----
    if do_x:
        xPP12 = sc(); tt(xPP12[:, XL], xB1[:, XL], xB2S[:, XL], ALU.mult)
        xPP01 = sc(); tt(xPP01[:, XL], xB0S[:, XL], xB1[:, XL], ALU.mult)
        xPP02 = sc(); tt(xPP02[:, XL], xB0S[:, XL], xB2S[:, XL], ALU.mult)
        sc.free(xB1, xB0S, xB2S)
        xg0L = sc(); tt(xg0L[:, XL], xPP12[:, XL], xdl0L[:, XL], ALU.mult)
        xg1L = sc(); tt(xg1L[:, XL], xPP02[:, XL], xdl1L[:, XL], ALU.mult)
        xg2L = sc(); tt(xg2L[:, XL], xPP01[:, XL], xdl2L[:, XL], ALU.mult)
        sc.free(xdl0L, xdl1L, xdl2L)
        xg0R = sc(); tt(xg0R[:, XL], xPP01[:, XL], xdl0R[:, XL], ALU.mult)
        xg1R = sc(); tt(xg1R[:, XL], xPP02[:, XL], xdl1R[:, XL], ALU.mult)
        xg2R = sc(); tt(xg2R[:, XL], xPP12[:, XL], xdl2R[:, XL], ALU.mult)
        sc.free(xdl0R, xdl1R, xdl2R)
        xdp = pe_acc([(xPP12, I1), (xPP02, I6), (xPP01, I3)], 2)
        xlnL = sc(); act(xlnL[:, 2:1026], xdp[:, 0:1024], AF.Ln)
        psc.free(xdp)
        xrdL = sc(); act(xrdL[:, 2:1026], xlnL[:, 2:1026], AF.Exp, bias=LN56, scale=-1.0)
        sc.free(xlnL)
        xdp = pe_acc([(xPP01, I1), (xPP02, I6), (xPP12, I3)], 2)
        xlnR = sc(); act(xlnR[:, 2:1026], xdp[:, 0:1024], AF.Ln)
        psc.free(xdp)
        xrdR = sc(); act(xrdR[:, 2:1026], xlnR[:, 2:1026], AF.Exp, bias=LN56, scale=-1.0)
        sc.free(xlnR)

    # ---- y: gammas, num, reconstruction, flux ----
    if do_y:
        sc.free(yPP12, yPP01, yPP02)
        yg0R = sc(); tt(yg0R[:, EY], yPP01p1[:, EY], ydl0R[:, EY], ALU.mult)
        yg1R = sc(); tt(yg1R[:, EY], yPP02p1[:, EY], ydl1R[:, EY], ALU.mult)
        yg2R = sc(); tt(yg2R[:, EY], yPP12p1[:, EY], ydl2R[:, EY], ALU.mult)
        sc.free(yPP01p1, yPP02p1, yPP12p1, ydl0R, ydl1R, ydl2R)
        yrdRp1 = pecopy(yrdR, SHP1)
        sc.free(yrdR)
        ynLp = pe_acc([(yg0L, I1), (yg1L, I1), (yg2L, I1)], 4)
        sc.free(yg0L, yg1L, yg2L)
        ytL = scf(); tt(ytL[:, EY], ynLp[:, 0:1024], yrdL[:, EY], ALU.mult)
        psc.free(ynLp)
        yrL = scf(); tt(yrL[:, EY], Q[:, EY], ytL[:, EY], ALU.add)
        sc.free(yrdL); scf.free(ytL)
        ynRp = pe_acc([(yg0R, I1), (yg1R, I1), (yg2R, I1)], 4)
        sc.free(yg0R, yg1R, yg2R)
        ytR = scf(); tt(ytR[:, EY], ynRp[:, 0:1024], yrdRp1[:, EY], ALU.mult)
        psc.free(ynRp)
        yrR = scf(); tt(yrR[:, EY], yqs1[:, EY], ytR[:, EY], ALU.subtract)
        sc.free(yrdRp1, yqs1); scf.free(ytR)
        # relu(V), relu(-V): tensor_scalar runs 2x_2P on DVE
        ypV = scf(); nc.vector.tensor_scalar_max(ypV[:, EY], Vf[:, EY], 0.0)
        ypVm = scf(); nc.vector.tensor_scalar(
            ypVm[:, EY], Vf[:, EY], -1.0, 0.0, ALU.mult, ALU.max)
        yaa = scf(); tt(yaa[:, EY], ypV[:, EY], yrL[:, EY], ALU.mult)
        scf.free(yrL, ypV)
        ybb = scf(); tt(ybb[:, EY], ypVm[:, EY], yrR[:, EY], ALU.mult)
        scf.free(ypVm, yrR)
        fn = scf(); tt(fn[:, EY], yaa[:, EY], ybb[:, EY], ALU.subtract)
        scf.free(yaa, ybb)
        pdfny = pe(fn, 0, bsrc=dfy32)
        scf.free(fn)

    # ---- x: num, reconstruction, flux (window XL) ----
    if do_x:
        sc.free(xPP12, xPP01, xPP02)
        xnLp = pe_acc([(xg0L, I1), (xg1L, I1), (xg2L, I1)], 2)
        sc.free(xg0L, xg1L, xg2L)
        xtL = scf(); tt(xtL[:, XL], xnLp[:, 2:1024], xrdL[:, XL], ALU.mult)
        psc.free(xnLp)
        xrL = scf(); tt(xrL[:, XL], Q[:, XL], xtL[:, XL], ALU.add)
        sc.free(xrdL); scf.free(xtL)
        xnRp = pe_acc([(xg0R, I1), (xg1R, I1), (xg2R, I1)], 2)
        sc.free(xg0R, xg1R, xg2R)
        xtR = scf(); tt(xtR[:, XL], xnRp[:, 2:1024], xrdR[:, XL], ALU.mult)
        psc.free(xnRp)
        xrR = scf(); tt(xrR[:, XL], Q[:, XL], xtR[:, XL], ALU.subtract)
        sc.free(xrdR); scf.free(xtR)
        xrRS = scf(); nc.sync.dma_start(xrRS[:, XL], xrR[:, 5:1027])
        scf.free(xrR)
        # relu(U), relu(-U): tensor_scalar runs 2x_2P on DVE
        xpU = scf(); nc.vector.tensor_scalar_max(xpU[:, XL], Uf[:, XL], 0.0)
        xpUm = scf(); nc.vector.tensor_scalar(
            xpUm[:, XL], Uf[:, XL], -1.0, 0.0, ALU.mult, ALU.max)
        xaa = scf(); tt(xaa[:, XL], xpU[:, XL], xrL[:, XL], ALU.mult)
        scf.free(xrL, xpU)
        xbb = scf(); tt(xbb[:, XL], xpUm[:, XL], xrRS[:, XL], ALU.mult)
        scf.free(xpUm, xrRS)
        fe = scf(); tt(fe[:, XL], xaa[:, XL], xbb[:, XL], ALU.subtract)
        scf.free(xaa, xbb)
        feS = scf(); nc.sync.dma_start(feS[:, 5:1026], fe[:, 4:1025])

    if full:
        z1 = scf()
        tt(z1[:, EY], feS[:, EY], pdfny[:, 0:1024], ALU.add)
        psc.free(pdfny)
        scf.free(feS)
        tt(oc2[:, XL], z1[:, XL], fe[:, XL], ALU.subtract)
        scf.free(z1, fe)
    elif mode == "xonly":
        tt(oc2[:, XL], feS[:, XL], fe[:, XL], ALU.subtract)
        scf.free(fe, feS)
    else:  # yonly
        act(oc2[:, EY], pdfny[:, 0:1024], AF.Copy)
        psc.free(pdfny)


def build_nc(zpc=ZPC, n_chunks=9, mode="full", repeat=1):
    nc = bass.Bass()
    # Exp's bias rides a const AP; LN56 isn't in the default database.
    _c = nc.alloc_sbuf_tensor("const-f32-ln56", [128, 1], F32)
    nc.gpsimd.memset(_c.ap(), LN56)
    nc.const_aps.aps[(F32, LN56)] = _c.ap()
    _e = nc.alloc_sbuf_tensor("const-f32-eps", [128, 1], F32)
    nc.gpsimd.memset(_e.ap(), WENO_EPS)
    nc.const_aps.aps[(F32, WENO_EPS)] = _e.ap()
    nc.all_engine_barrier()
    h_ext = nc.declare_dram_parameter("h", [zpc, PY, PX], BF16, isOutput=False)
    u_ext = nc.declare_dram_parameter("u", [zpc, PY, PX], F32, isOutput=False)
    v_ext = nc.declare_dram_parameter("v", [zpc, PY, PX], F32, isOutput=False)
    b_ext = nc.declare_dram_parameter(
        "bands", [128, NBANDS * 128], BF16, isOutput=False
    )
    d_ext = nc.declare_dram_parameter("dfy", [128, 128], F32, isOutput=False)
    o_ext = nc.declare_dram_parameter("o", [zpc, NY, NX], F32, isOutput=True)

    with LegalTileContext(nc) as tc:
        with (
            tc.tile_pool(name="inp", bufs=2) as inp,
            tc.tile_pool(name="wk", bufs=2) as wk,
            tc.tile_pool(name="wkf", bufs=2) as wkf,
            tc.tile_pool(name="outp", bufs=2) as outp,
            tc.tile_pool(name="bnd", bufs=1) as bnd,
            tc.tile_pool(name="ps", bufs=2, space="PSUM") as psum,
        ):
            bands = bnd.tile([128, NBANDS * 128], BF16, tag="bands")
            nc.sync.dma_start(bands[:], b_ext[:])
            dfy32 = bnd.tile([128, 128], F32, tag="dfy")
            nc.sync.dma_start(dfy32[:], d_ext[:])
            sc = Scratch(wk, [128, W], BF16)
            scf = Scratch(wkf, [128, W], F32, prefix="f")
            psc = Scratch(psum, [128, 1024], F32, prefix="p")
            for _rep in range(repeat):
              for z in range(zpc):
                for ci in range(n_chunks):
                    r0 = CHUNK * ci
                    if r0 + 128 > PY:
                        r0 = PY - 128
                    Q = inp.tile([128, W], BF16, tag="Q")
                    nc.sync.dma_start(Q[:, 2:1028], h_ext[z, r0 : r0 + 128, :])
                    Uf = inp.tile([128, W], F32, tag="U")
                    nc.sync.dma_start(Uf[:, 2:1028], u_ext[z, r0 : r0 + 128, :])
                    Vf = inp.tile([128, W], F32, tag="V")
                    nc.sync.dma_start(Vf[:, 2:1028], v_ext[z, r0 : r0 + 128, :])

                    oc2 = outp.tile([128, W], F32, tag="oc2")
                    _emit_chunk(
                        nc, sc, scf, psc, bands, dfy32, Q, Uf, Vf, oc2, mode
                    )
                    # tile col t -> global x = t - 3; rows p in [3..124]
                    gy0 = r0 + 2
                    nc.sync.dma_start(
                        o_ext[z, gy0 : gy0 + 122, 2 : NX - 2],
                        oc2[3:125, 5:1025],
                    )
    import sys
    print(
        f"build_nc: scratch_tags={sc.n} f32_tags={scf.n} psum_tags={psc.n}",
        file=sys.stderr,
    )
    return nc


_nc_cache = {}


def _get_nc(zpc=ZPC, n_chunks=9, mode="full", repeat=1):
    key = (zpc, n_chunks, mode, repeat)
    if key not in _nc_cache:
        _nc_cache[key] = build_nc(zpc, n_chunks, mode, repeat)
    return _nc_cache[key]


def _levels():
    # z-levels 1..30 need computing; pad to 8*4 with repeats of level 30
    return list(range(1, NZ - 1)) + [NZ - 2, NZ - 2]


def make_in_maps(h, u, v):
    import ml_dtypes

    h = np.asarray(h, dtype=np.float32)
    u = np.asarray(u, dtype=np.float32)
    v = np.asarray(v, dtype=np.float32)
    hp = np.pad(h, ((0, 0), (1, 1), (1, 1)), mode="edge").astype(ml_dtypes.bfloat16)
    up = np.pad(u, ((0, 0), (1, 1), (1, 1)), mode="edge") * np.float32(1.0 / DX)
    vp = np.pad(v, ((0, 0), (1, 1), (1, 1)), mode="edge") * np.float32(1.0 / DY)
    levels = _levels()
    bands = make_bands_host()
    dfy = make_dfy_host()
    in_maps = []
    for c in range(NCORES):
        lv = levels[c * ZPC : (c + 1) * ZPC]
        in_maps.append(
            {
                "h": np.ascontiguousarray(hp[lv]),
                "u": np.ascontiguousarray(up[lv]),
                "v": np.ascontiguousarray(vp[lv]),
                "bands": bands,
                "dfy": dfy,
            }
        )
    return in_maps


def kernel(h, u, v):
    from concourse.bass_utils import run_bass_kernel_spmd

    nc = _get_nc()
    core_ids = list(range(NCORES))
    in_maps = make_in_maps(h, u, v)
    res = run_bass_kernel_spmd(nc, in_maps, core_ids)
    levels = _levels()
    out = np.zeros((NZ, NY, NX), dtype=np.float32)
    for c in core_ids:
        lv = levels[c * ZPC : (c + 1) * ZPC]
        o = res.results[c]["o"]
        for j, z in enumerate(lv):
            out[z, 2 : NY - 2, 2 : NX - 2] = o[j][2 : NY - 2, 2 : NX - 2]
    return out
